# revision 1
# baseline (speedup 1.0000x reference)
"""Patch-embedding kernel for Trainium2, data-parallel over batch on 8 NeuronCores.

Reference computation (per image):
  patches = im2col(image, 16x16)            # [196, 768]
  out = gelu(patches @ W + b, exact)        # [196, 768]

Sharding: batch 64 -> 8 images per core. Each core runs an identical NEFF on
its own slice; host concatenates the per-core outputs.

Per-core pipeline (M = 8*196 = 1568 patch rows, K = N = 768, m-tile = 112
= 8 whole patch-rows so tiles never split a patch-row):
  1. im2col: HWDGE DMAs (sync ring) gather whole patch-row segments into
     patch-major SBUF tiles X_f32 [112, 768].
  2. DVE cast f32 -> bf16.
  3. One xbar DMA transpose per m-tile (scalar ring): [112, 768] ->
     [128, 6, 112]; output dims (p, kc) jointly hold the transposed k index
     as k = 6*p + kc, which is mirrored by loading W with the
     "(p kc) n -> p kc n" rearrange so the contraction pairs line up.
  4. PE matmul, bf16 inputs, fp32 PSUM accumulation over 6 k-chunks;
     bias is applied with a rank-1 (K=1) matmul that initializes PSUM.
  5. Exact GELU on ScalarE, PSUM -> SBUF f32.
  6. Contiguous store to DRAM (sync ring).

Keeping all transpose-mode DMAs on the scalar HWDGE ring and all copy-mode
DMAs on the sync ring avoids the xbar-mode transition serialization.
"""

import numpy as np
import ml_dtypes

import concourse.bass as bass
import concourse.tile as tile
import concourse.mybir as mybir
from concourse import bacc
from concourse.bass_utils import run_bass_kernel_spmd

P = 16
D = 768
B, H, W, C = 64, 224, 224, 3
NH = NW = 14
NPATCH = NH * NW            # 196
K = P * P * C               # 768
NCORES = 8
BPC = B // NCORES           # 8 images per core
M = BPC * NPATCH            # 1568 output rows per core
MT = 112                    # m-tile: 8 whole patch-rows
NMT = M // MT               # 14 tiles, all full
KC = K // 128               # 6 k-chunks

_BF16 = mybir.dt.bfloat16
_F32 = mybir.dt.float32


def _m_segments(t):
    """Whole-patch-row runs inside m-tile t that stay within one image.

    Yields (p0, b, i0, nrows): partitions p0..p0+nrows*14 map to patch rows
    i0..i0+nrows of image b.
    """
    m0 = t * MT
    m1 = m0 + MT
    m = m0
    while m < m1:
        b, r = divmod(m, NPATCH)
        i = r // NW
        assert r % NW == 0
        nrows = min(NPATCH - r, m1 - m) // NW
        yield (m - m0, b, i, nrows)
        m += nrows * NW


def _build(transpose_mode="xbar"):
    nc = bacc.Bacc("TRN2", target_bir_lowering=False, debug=False,
                   num_devices=NCORES)
    img = nc.dram_tensor("img", [BPC, H, W, C], _F32, kind="ExternalInput").ap()
    w = nc.dram_tensor("w", [K, D], _BF16, kind="ExternalInput").ap()
    bias = nc.dram_tensor("bias", [1, D], _BF16, kind="ExternalInput").ap()
    # Sub-tile-major output: [patch-row i, image b, patch-col j, D]. Each
    # sub-tile store is one flat contiguous [112, 768] write (a 2-level
    # partition AP on the SBUF side is mishandled by the DMA path); the host
    # transposes (i, b) back.
    out = nc.dram_tensor("out", [NH, BPC, NW, D], _F32, kind="ExternalOutput").ap()

    with tile.TileContext(nc) as tc:
        _body(tc, img, w, bias, out, transpose_mode)
    nc.compile()
    return nc


def _body(tc, img, w, bias, out, transpose_mode):
    import contextlib
    ctx = contextlib.ExitStack()
    with ctx:
        nc = tc.nc
        singles = ctx.enter_context(tc.tile_pool(name="singles", bufs=1))
        xpool = ctx.enter_context(tc.tile_pool(name="x", bufs=2))
        xtpool = ctx.enter_context(tc.tile_pool(name="xt", bufs=2))
        opool = ctx.enter_context(tc.tile_pool(name="o", bufs=3))
        pspool = ctx.enter_context(tc.tile_pool(name="ps", bufs=4, space="PSUM"))

        # Stationary weights, chunked k layout: w_sb[p, kc, n] = W[128*kc+p, n].
        # Matches both the 3D-out xbar transpose (row q = 128*kc + p,
        # verified on HW) and the per-chunk PE transpose.
        w_sb = singles.tile([128, KC, D], _BF16)
        nc.sync.dma_start(out=w_sb[:],
                          in_=w.rearrange("(kc p) n -> p kc n", p=128))
        bias_sb = singles.tile([1, D], _BF16)
        nc.sync.dma_start(out=bias_sb[:], in_=bias[:])
        ones_sb = singles.tile([1, MT], _BF16)
        nc.vector.memset(ones_sb[:], 1.0)

        if transpose_mode == "pe":
            ident = singles.tile([MT, MT], _BF16)
            from concourse.masks import make_identity
            make_identity(nc, ident[:])

        # Block layout. Partitions = (image, patch-col) = 8*14 = 112; the
        # free dim stacks a block of patch-rows vertically: k2 = s_loc*768 + k
        # where k = 48*ph + u. One im2col DMA per image per block (3-dim AP:
        # [j, image-row, 48-contig], max_dma_last_dim=48 keeps the efficient
        # per-fragment 192B descriptors), and ONE xbar transpose per block
        # (free dim is a multiple of 128). Blocks of 4/4/4/2 patch-rows give
        # a 4-deep pipeline.
        for (i0, n_i) in [(0, 4), (4, 4), (8, 4), (12, 2)]:
            kf = n_i * K
            x_f32 = xpool.tile([MT, 4 * K], _F32, tag="x_f32")
            for b in range(BPC):
                src = img[b, P * i0:P * (i0 + n_i), :, :]
                src = src.rearrange("r (j pw) c -> j r (pw c)", pw=P)
                dst = x_f32[b * NW:(b + 1) * NW, :kf].rearrange(
                    "j (r u) -> j r u", u=P * C)
                nc.sync.dma_start(out=dst, in_=src, max_dma_last_dim=P * C)

            x_bf = xpool.tile([MT, 4 * K], _BF16, tag="x_bf")
            nc.vector.tensor_copy(x_bf[:, :kf], x_f32[:, :kf])

            xt = xtpool.tile([128, 4 * KC, MT], _BF16, tag="xt")
            if transpose_mode == "xbar":
                nc.scalar.dma_start(out=xt[:, :n_i * KC, :],
                                    in_=x_bf[:, :kf], transpose=True)
            else:
                for g in range(n_i * KC):
                    ps_t = pspool.tile([128, MT], _BF16, tag="ps_t")
                    nc.tensor.transpose(
                        ps_t[:], x_bf[:, 128 * g:128 * (g + 1)], ident[:])
                    nc.vector.tensor_copy(xt[:, g, :], ps_t[:])

            for s in range(n_i):
                i = i0 + s
                psum = pspool.tile([MT, D], _F32, tag="psum")
                # Rank-1 bias matmul initializes the accumulation.
                nc.tensor.matmul(psum[:, 0:512], ones_sb[:, :],
                                 bias_sb[:, 0:512], start=True, stop=False)
                nc.tensor.matmul(psum[:, 512:D], ones_sb[:, :],
                                 bias_sb[:, 512:D], start=True, stop=False)
                for kc in range(KC):
                    last = kc == KC - 1
                    g = s * KC + kc
                    nc.tensor.matmul(psum[:, 0:512], xt[:, g, :],
                                     w_sb[:, kc, 0:512], start=False,
                                     stop=last)
                    nc.tensor.matmul(psum[:, 512:D], xt[:, g, :],
                                     w_sb[:, kc, 512:D], start=False,
                                     stop=last)

                o_sb = opool.tile([MT, D], _F32)
                nc.scalar.activation(o_sb[:], psum[:],
                                     mybir.ActivationFunctionType.Gelu)
                # partition (b, j) -> out[i, b, j, :], flat contiguous store
                nc.sync.dma_start(
                    out=out[i].rearrange("b j d -> (b j) d"), in_=o_sb[:])


_NC_CACHE = {}


def _get_nc(transpose_mode="xbar"):
    if transpose_mode not in _NC_CACHE:
        _NC_CACHE[transpose_mode] = _build(transpose_mode)
    return _NC_CACHE[transpose_mode]


def _run(image, W_proj, b_proj, transpose_mode="xbar", **spmd_kwargs):
    image = np.asarray(image, dtype=np.float32)
    w_bf = np.asarray(W_proj).astype(ml_dtypes.bfloat16)
    b_bf = np.asarray(b_proj).astype(ml_dtypes.bfloat16).reshape(1, D)
    assert image.shape == (B, H, W, C)

    nc = _get_nc(transpose_mode)
    in_maps = [
        {"img": np.ascontiguousarray(image[c * BPC:(c + 1) * BPC]),
         "w": w_bf, "bias": b_bf}
        for c in range(NCORES)
    ]
    res = run_bass_kernel_spmd(nc, in_maps, core_ids=list(range(NCORES)),
                               **spmd_kwargs)
    # device layout [i, b, j, D] -> [b, (i j), D]
    outs = [res.results[c]["out"].transpose(1, 0, 2, 3).reshape(BPC, NPATCH, D)
            for c in range(NCORES)]
    full = np.concatenate(outs, axis=0).astype(np.float32)
    return full, res


def kernel(image, W_proj, b_proj):
    full, _ = _run(image, W_proj, b_proj)
    return full



# revision 2
# speedup vs baseline: 3.0940x; 3.0940x over previous
"""Patch-embedding kernel for Trainium2, data-parallel over batch on 8 NeuronCores.

Reference computation (per image):
  patches = im2col(image, 16x16)            # [196, 768]
  out = gelu(patches @ W + b, exact)        # [196, 768]

Sharding: batch 64 -> 8 images per core; host concatenates per-core outputs.

Layout strategy: im2col is a pure permutation for stride-16 non-overlapping
patches, so the HOST performs im2col + transpose + bf16 cast and uploads
X^T in k-major chunk layout xt[p, kc, m] = X[m, 128*kc + p]. Every device
DMA is then a wide contiguous read (>=1.5 KB per partition line) -- this
removes the 192-byte-fragment im2col gather and the on-device xbar
transposes that dominated the previous version.

Matmul orientation: transposed output. For each 128-wide n-chunk,
  psum[n, m] = sum_kc W[k, n].T @ X^T[k, m]
with W chunks as the stationary operand (natural layout, uploaded
pre-chunked) and X^T as the bf16 moving operand. Benefits:
  - no bias matmuls: bias is per-PARTITION in this orientation, applied for
    free by ScalarE as gelu(psum + bias[p]) during the PSUM->SBUF pass
  - m-tiles of 392 (=1568/4) tile M exactly; lhsT is always full 128x128
  - output stored bf16 (halves store traffic); host transposes + upcasts.

Per-core loop: for each m-tile (392 rows), for each of 6 n-chunks,
accumulate 6 k-chunk matmuls into one PSUM bank, then ScalarE applies
exact GELU (+bias) writing bf16, and the result is DMA'd out. 24 PSUM
groups rotate through all 8 banks; X^T arrives in 12 half-chunks so the
first m-tile's matmuls start after ~1.8 MB of loads instead of 3.6 MB.
"""

import numpy as np
import ml_dtypes

import concourse.bass as bass
import concourse.tile as tile
import concourse.mybir as mybir
from concourse import bacc
from concourse.bass_utils import run_bass_kernel_spmd

P = 16
D = 768
B, H, W, C = 64, 224, 224, 3
NH = NW = 14
NPATCH = NH * NW            # 196
K = P * P * C               # 768
NCORES = 8
BPC = B // NCORES           # 8 images per core
M = BPC * NPATCH            # 1568 output rows per core
KC = K // 128               # 6 k-chunks
NC6 = D // 128              # 6 n-chunks
MT = 392                    # m-tile (1568 = 4*392)
NMT = M // MT               # 4 m-tiles
MH = M // 2                 # 784, half-M granularity for the X^T loads

_BF16 = mybir.dt.bfloat16
_F32 = mybir.dt.float32


def _build():
    nc = bacc.Bacc("TRN2", target_bir_lowering=False, debug=False,
                   num_devices=NCORES)
    # Host-prepared layouts (see _run): all reads/writes contiguous.
    xt = nc.dram_tensor("xt", [128, KC, M], _BF16, kind="ExternalInput").ap()
    w = nc.dram_tensor("w", [128, KC, D], _BF16, kind="ExternalInput").ap()
    bias = nc.dram_tensor("bias", [128, NC6], _F32, kind="ExternalInput").ap()
    # Transposed output out[p, n6, m] = result[m, 128*n6 + p]; host unscrambles.
    out = nc.dram_tensor("out", [128, NC6, M], _BF16, kind="ExternalOutput").ap()

    with tile.TileContext(nc) as tc:
        _body(tc, xt, w, bias, out)
    nc.compile()
    return nc


def _body(tc, xt, w, bias, out):
    import contextlib
    ctx = contextlib.ExitStack()
    with ctx:
        nc = tc.nc
        singles = ctx.enter_context(tc.tile_pool(name="singles", bufs=1))
        opool = ctx.enter_context(tc.tile_pool(name="o", bufs=4))
        pspool = ctx.enter_context(tc.tile_pool(name="ps", bufs=8, space="PSUM"))

        bias_sb = singles.tile([128, NC6], _F32)
        nc.sync.dma_start(out=bias_sb[:], in_=bias[:])
        w_sb = singles.tile([128, KC, D], _BF16)
        nc.sync.dma_start(out=w_sb[:], in_=w[:])
        # X^T in 12 half-M chunks so early m-tiles can start before the
        # whole activation tensor is resident.
        xt_sb = singles.tile([128, KC, M], _BF16)
        for h in range(2):
            for kc in range(KC):
                sl = np.s_[:, kc, h * MH:(h + 1) * MH]
                nc.sync.dma_start(out=xt_sb[sl], in_=xt[sl])

        for mt in range(NMT):
            m0 = mt * MT
            for n6 in range(NC6):
                ps = pspool.tile([128, 512], _F32, tag="ps")
                for kc in range(KC):
                    nc.tensor.matmul(ps[:, :MT],
                                     w_sb[:, kc, n6 * 128:(n6 + 1) * 128],
                                     xt_sb[:, kc, m0:m0 + MT],
                                     start=(kc == 0), stop=(kc == KC - 1))
                o_sb = opool.tile([128, MT], _BF16, tag="o")
                nc.scalar.activation(o_sb[:], ps[:, :MT],
                                     mybir.ActivationFunctionType.Gelu,
                                     bias=bias_sb[:, n6:n6 + 1])
                nc.sync.dma_start(out=out[:, n6, m0:m0 + MT], in_=o_sb[:])


_NC_CACHE = {}


def _get_nc():
    if "nc" not in _NC_CACHE:
        _NC_CACHE["nc"] = _build()
    return _NC_CACHE["nc"]


def _prep_core_inputs(image, W_proj, b_proj):
    """Host-side layout prep: im2col + transpose + bf16, all permutations."""
    image = np.asarray(image, dtype=np.float32)
    assert image.shape == (B, H, W, C)
    img_bf = image.astype(ml_dtypes.bfloat16)
    # im2col (row-major patch order, matching the reference)
    pat = img_bf.reshape(B, NH, P, NW, P, C).transpose(0, 1, 3, 2, 4, 5)
    pat = np.ascontiguousarray(pat).reshape(B, NPATCH, K)

    w_bf = np.asarray(W_proj, dtype=np.float32).astype(ml_dtypes.bfloat16)
    w_dev = np.ascontiguousarray(w_bf.reshape(KC, 128, D).transpose(1, 0, 2))
    b_dev = np.ascontiguousarray(
        np.asarray(b_proj, dtype=np.float32).reshape(NC6, 128).T)

    in_maps = []
    for c in range(NCORES):
        x = pat[c * BPC:(c + 1) * BPC].reshape(M, K)
        # xt[p, kc, m] = x[m, 128*kc + p]
        xt = np.ascontiguousarray(x.reshape(M, KC, 128).transpose(2, 1, 0))
        in_maps.append({"xt": xt, "w": w_dev, "bias": b_dev})
    return in_maps


def _run(image, W_proj, b_proj, **spmd_kwargs):
    spmd_kwargs.pop("transpose_mode", None)
    in_maps = _prep_core_inputs(image, W_proj, b_proj)
    nc = _get_nc()
    res = run_bass_kernel_spmd(nc, in_maps, core_ids=list(range(NCORES)),
                               **spmd_kwargs)
    # device layout [p, n6, m] -> [m, 128*n6+p] -> [BPC, NPATCH, D] f32
    outs = [
        np.ascontiguousarray(res.results[c]["out"].transpose(2, 1, 0))
        .astype(np.float32).reshape(BPC, NPATCH, D)
        for c in range(NCORES)
    ]
    full = np.concatenate(outs, axis=0)
    return full, res


def kernel(image, W_proj, b_proj):
    full, _ = _run(image, W_proj, b_proj)
    return full
